# revision 1
# baseline (speedup 1.0000x reference)
"""Bayesian-router MoE kernel for 8 Trainium2 NeuronCores.

Strategy (expert-parallel, per sharding hint):
  - Router moments / top-k / combine weights: tiny (B*F*E ~ 17 MFLOP), computed
    on host in float64 (min rank4/rank5 score gap is ~1.7e-4, far above fp32
    noise, so expert selection is stable vs the fp32 reference).
  - Token dispatch: host gathers each expert's routed tokens into a padded,
    transposed buffer XgT [F, CAP] (the host-side equivalent of the
    all-to-all; full I/O contract means shard/unshard happens on host).
    Experts are sorted by token count: the 8 largest go to slot 0 (cap0),
    the 8 smallest to slot 1 (cap1 <= cap0), one of each per core, so the
    SPMD program wastes less padding compute.
  - Device: each of the 8 cores runs its 2-expert MLP on gathered tokens,
    entirely in transposed form (A1T = relu(W1^T XgT + b1), YT = W2^T A1T + b2)
    so no on-device transposes are needed; weights stream as lhsT directly.
    Tokens/weights/activations are shipped and multiplied as fp16 (PSUM
    accumulation stays fp32), which halves input DMA vs fp32 and runs the
    PE at 1 cycle/row -- 4x the plain-fp32 matmul rate, with only the
    ~5e-4 input-rounding error (measured end-to-end rel err 4.2e-4).
  - Combine: host scatter-adds w[t,e] * Y_e rows into the output (the
    cross-device reduction of the unshard step).
"""

import os
import numpy as np

NCORES = 8
P = 128
TOP_K = 4


# ---------------------------------------------------------------------------
# host-side routing (matches reference math; float64 for stable ordering)
# ---------------------------------------------------------------------------
def _routing(h, W_mu, b_mu, W_logvar, b_logvar):
    h64 = h.astype(np.float64)
    mu = h64 @ W_mu.T.astype(np.float64) + b_mu.astype(np.float64)
    var = (h64 * h64) @ np.exp(W_logvar.astype(np.float64)).T + np.exp(
        b_logvar.astype(np.float64)
    )
    var = np.maximum(var, 1e-12)
    tilde = mu / np.sqrt(1.0 + (np.pi / 8.0) * var)
    t = tilde - tilde.max(axis=1, keepdims=True)
    ex = np.exp(t)
    probs = ex / ex.sum(axis=1, keepdims=True)
    idx = np.argsort(-tilde, axis=1, kind="stable")[:, :TOP_K]
    w = np.take_along_axis(probs, idx, axis=1)
    w = w / np.maximum(w.sum(axis=1, keepdims=True), 1e-12)
    return idx, w


def _chunks(cap):
    n = (cap + 511) // 512
    base, rem = divmod(cap, n)
    out = []
    off = 0
    for i in range(n):
        sz = base + (1 if i < rem else 0)
        out.append((off, sz))
        off += sz
    return out


# ---------------------------------------------------------------------------
# device kernel: 2-expert MLP on pre-gathered transposed tokens
# ---------------------------------------------------------------------------
def _build_kernel(F, H, C, caps):
    import concourse.mybir as mybir
    import concourse.tile as tile
    from concourse import bacc

    f32 = mybir.dt.float32
    f16 = mybir.dt.float16
    FK, HK, CK = F // P, H // P, C // P
    nslots = len(caps)

    nc = bacc.Bacc("TRN2", target_bir_lowering=False, debug=False,
                   num_devices=NCORES)

    # tokens/weights/activations ship and compute as fp16 (PSUM accumulation
    # stays fp32, so the only precision loss is the input rounding ~5e-4)
    xts_d = [nc.dram_tensor(f"xt{s}", [F, caps[s]], f16, kind="ExternalInput")
             for s in range(nslots)]
    yts_d = [nc.dram_tensor(f"yt{s}", [C, caps[s]], f32, kind="ExternalOutput")
             for s in range(nslots)]
    w1 = nc.dram_tensor("w1", [nslots, F, H], f16, kind="ExternalInput")
    w2 = nc.dram_tensor("w2", [nslots, H, C], f16, kind="ExternalInput")
    b1 = nc.dram_tensor("b1", [P, nslots, HK], f32, kind="ExternalInput")
    b2 = nc.dram_tensor("b2", [P, nslots, CK], f32, kind="ExternalInput")

    with tile.TileContext(nc) as tc:
        with (
            tc.tile_pool(name="consts", bufs=1) as consts,
            tc.tile_pool(name="w1pool", bufs=2) as w1pool,
            tc.tile_pool(name="w2pool", bufs=2) as w2pool,
            tc.tile_pool(name="xpool", bufs=2) as xpool,
            tc.tile_pool(name="apool", bufs=2) as apool,
            tc.tile_pool(name="ypool", bufs=2) as ypool,
            tc.tile_pool(name="psum", bufs=8, space="PSUM") as pp,
        ):
            b1s = consts.tile([P, nslots, HK], f32)
            nc.gpsimd.dma_start(out=b1s[:], in_=b1[:])
            b2s = consts.tile([P, nslots, CK], f32)
            nc.gpsimd.dma_start(out=b2s[:], in_=b2[:])

            add, amax = mybir.AluOpType.add, mybir.AluOpType.max

            def evict(i, dst, src, bias, relu):
                # alternate PSUM evictions between Scalar(ACT) and Vector(DVE)
                # so neither engine falls behind the matmul stream
                if i % 2 == 0:
                    return nc.scalar.activation(
                        dst, src,
                        mybir.ActivationFunctionType.Relu if relu
                        else mybir.ActivationFunctionType.Identity,
                        bias=bias,
                    )
                elif relu:
                    return nc.vector.tensor_scalar(dst, src, bias, 0.0, add,
                                                   amax)
                else:
                    return nc.vector.tensor_scalar_add(dst, src, bias)

            # SDMA engines round-robin across the queued transfers, so a
            # later-needed bulk transfer queued early steals bandwidth from
            # (and delays the completion of) the ramp-critical xt/w1 of the
            # first slot. gate_after delays those transfers behind an early
            # eviction of the preceding phase.
            l1_evs = []
            l2_evs = []

            def gate_after(dma_binst, gate_inst):
                if gate_inst is not None:
                    tile.add_dep_helper(
                        dma_binst.ins, gate_inst.ins,
                        reason="delay bulk DMA past the ramp-critical phase",
                    )

            for s in range(nslots):
                cap = caps[s]
                chunks = _chunks(cap)
                # tokens on the scalar HWDGE ring, weights on the sync ring;
                # slot-0 transfers split just enough that the first matmul
                # groups' operands land early (each dma_start costs ~600ns of
                # ring issue time, so over-splitting backfires)
                xts = xpool.tile([P, FK, cap], f16, tag=f"xt{s}")
                xt_r = xts_d[s].rearrange("(k p) n -> p k n", p=P)
                if s == 0:
                    nc.scalar.dma_start(out=xts[:, :2], in_=xt_r[:, :2])
                    nc.scalar.dma_start(out=xts[:, 2:], in_=xt_r[:, 2:])
                else:
                    gate_after(
                        nc.scalar.dma_start(out=xts[:], in_=xt_r[:]),
                        l1_evs[3] if len(l1_evs) > 3 else None,
                    )
                w1s = w1pool.tile([P, FK, H], f16, tag="w1")
                w1_r = w1[s].rearrange("(k p) m -> p k m", p=P)
                MW1 = H // 4
                if s == 0:
                    for j in range(4):
                        nc.sync.dma_start(
                            out=w1s[:, :, j * MW1:(j + 1) * MW1],
                            in_=w1_r[:, :, j * MW1:(j + 1) * MW1],
                        )
                else:
                    gate_after(
                        nc.sync.dma_start(out=w1s[:], in_=w1_r[:]),
                        l1_evs[3] if len(l1_evs) > 3 else None,
                    )
                w2s = w2pool.tile([P, HK, C], f16, tag="w2")
                w2_r = w2[s].rearrange("(k p) m -> p k m", p=P)
                if s == 0:
                    MW2 = C // 2
                    w2_dmas = [
                        nc.sync.dma_start(out=w2s[:, :, :MW2],
                                          in_=w2_r[:, :, :MW2]),
                        nc.sync.dma_start(out=w2s[:, :, MW2:],
                                          in_=w2_r[:, :, MW2:]),
                    ]
                else:
                    gate_after(
                        nc.sync.dma_start(out=w2s[:], in_=w2_r[:]),
                        l2_evs[0] if l2_evs else None,
                    )

                a1s = apool.tile([P, HK, cap], f16, tag="a1")
                ysb = ypool.tile([P, CK, cap], f32, tag="yt")
                yt_r = yts_d[s].rearrange("(k p) n -> p k n", p=P)

                # all layer-1 m-groups first, then all layer-2: PE has ready
                # work across the L1->L2 boundary and the slot seam
                ev = 0
                for m in range(HK):
                    for n0, nsz in chunks:
                        ps = pp.tile([P, 512], f32, tag="ps")
                        for k in range(FK):
                            nc.tensor.matmul(
                                ps[:, :nsz],
                                w1s[:, k, m * P:(m + 1) * P],
                                xts[:, k, n0:n0 + nsz],
                                start=(k == 0),
                                stop=(k == FK - 1),
                            )
                        e_inst = evict(ev, a1s[:, m, n0:n0 + nsz],
                                       ps[:, :nsz], b1s[:, s, m:m + 1],
                                       relu=True)
                        if s == 0:
                            l1_evs.append(e_inst)
                        ev += 1
                if s == 0:
                    # release slot-0 w2 transfers once layer 1 is underway
                    for dma in w2_dmas:
                        gate_after(dma, l1_evs[1])

                for m in range(CK):
                    for n0, nsz in chunks:
                        ps = pp.tile([P, 512], f32, tag="ps")
                        for k in range(HK):
                            nc.tensor.matmul(
                                ps[:, :nsz],
                                w2s[:, k, m * P:(m + 1) * P],
                                a1s[:, k, n0:n0 + nsz],
                                start=(k == 0),
                                stop=(k == HK - 1),
                            )
                        e_inst = evict(ev, ysb[:, m, n0:n0 + nsz],
                                       ps[:, :nsz], b2s[:, s, m:m + 1],
                                       relu=False)
                        if s == 0:
                            l2_evs.append(e_inst)
                        ev += 1
                        if n0 + nsz == cap:
                            # whole row of ysb done -> stream it out
                            nc.scalar.dma_start(out=yt_r[:, m], in_=ysb[:, m])

    nc.compile()
    return nc


# ---------------------------------------------------------------------------
# entry point
# ---------------------------------------------------------------------------
def kernel(h, W_mu, b_mu, W_logvar, b_logvar, W1, b1, W2, b2):
    from concourse.bass_utils import run_bass_kernel_spmd

    h = np.ascontiguousarray(np.asarray(h, dtype=np.float32))
    W1 = np.asarray(W1, dtype=np.float32)
    b1 = np.asarray(b1, dtype=np.float32)
    W2 = np.asarray(W2, dtype=np.float32)
    b2 = np.asarray(b2, dtype=np.float32)

    B, F = h.shape
    E, _, H = W1.shape
    C = W2.shape[2]
    assert E % NCORES == 0
    nslots = E // NCORES
    FK, HK, CK = F // P, H // P, C // P

    topk_idx, topk_w = _routing(
        np.asarray(h), np.asarray(W_mu), np.asarray(b_mu),
        np.asarray(W_logvar), np.asarray(b_logvar)
    )

    # per-expert token lists; sort experts by count so each slot's capacity
    # is the max within that slot (slot 0 = busiest experts)
    toks, poss = [], []
    counts = np.zeros(E, np.int64)
    for e in range(E):
        tok, pos = np.nonzero(topk_idx == e)
        toks.append(tok)
        poss.append(pos)
        counts[e] = len(tok)
    perm = np.argsort(-counts, kind="stable")
    caps = []
    for s in range(nslots):
        grp = perm[s * NCORES:(s + 1) * NCORES]
        caps.append(max(64, int(-(-counts[grp].max() // 32) * 32)))

    # gather/dispatch: XgT per expert, padded to its slot's cap
    xt = [np.zeros((NCORES, F, caps[s]), np.float16) for s in range(nslots)]
    w1_in = np.empty((NCORES, nslots, F, H), np.float16)
    w2_in = np.empty((NCORES, nslots, H, C), np.float16)
    b1_in = np.empty((NCORES, P, nslots, HK), np.float32)
    b2_in = np.empty((NCORES, P, nslots, CK), np.float32)
    for i, e in enumerate(perm):
        s, c = divmod(i, NCORES)
        xt[s][c, :, :counts[e]] = h[toks[e]].T.astype(np.float16)
        w1_in[c, s] = W1[e]
        w2_in[c, s] = W2[e]
        b1_in[c, :, s, :] = b1[e].reshape(HK, P).T
        b2_in[c, :, s, :] = b2[e].reshape(CK, P).T

    nc = _build_kernel(F, H, C, caps)

    in_maps = []
    for c in range(NCORES):
        m = {"w1": w1_in[c], "w2": w2_in[c], "b1": b1_in[c], "b2": b2_in[c]}
        for s in range(nslots):
            m[f"xt{s}"] = xt[s][c]
        in_maps.append(m)

    trace = bool(os.environ.get("MOE_KERNEL_TRACE"))
    res = run_bass_kernel_spmd(nc, in_maps, list(range(NCORES)), trace=trace)
    global LAST_RESULTS
    LAST_RESULTS = res

    # combine: scatter-add weighted expert outputs
    out = np.zeros((B, C), np.float32)
    for i, e in enumerate(perm):
        s, c = divmod(i, NCORES)
        cnt = counts[e]
        yte = res.results[c][f"yt{s}"]  # [C, cap_s]
        out[toks[e]] += (
            topk_w[toks[e], poss[e]].astype(np.float32)[:, None]
            * yte[:, :cnt].T
        )
    return out


LAST_RESULTS = None

